# revision 42
# baseline (speedup 1.0000x reference)
"""DiT self-attention Trainium2 kernel, 8-way head-parallel (tensor parallel).

Strategy (per spec sharding_hint):
  - QKV projections column-sharded over heads: each of the 8 cores computes
    its 2 heads (256 channels) for all B*S tokens.  RMSNorm needs full-row
    sum-of-squares -> tiny AllReduce of per-token partials ([2,S] f32/batch).
  - RoPE applied locally (channels permuted host-side so that real/imag
    halves live in separate 16-partition blocks, making the rotation a
    stream_shuffle + 2 mul + 1 add on DVE).
  - Attention per (batch, local head): S^T = K^T Q tiles -> exp on ACT ->
    P^T; PV with a ones column appended to V gives the softmax denominator
    for free (129th output column).
  - Attention outputs are resharded token-wise with an AllToAll per batch,
    then the output projection runs on the local 2*256 tokens with the full
    wo (row sharding), so no further reduction is needed.

All matmuls run in bf16 (fp32 PSUM accumulation); norms/softmax math fp32.
"""

import math
import os
import sys

for _p in ("/opt/trn_rl_repo",):
    if _p not in sys.path and os.path.isdir(_p):
        sys.path.insert(0, _p)

import ml_dtypes
import numpy as np

import concourse.bacc as bacc
import concourse.bass as bass
import concourse.mybir as mybir
import concourse.tile as tile
from concourse.bass_utils import run_bass_kernel_spmd

BF16 = mybir.dt.bfloat16
F32 = mybir.dt.float32
AF = mybir.ActivationFunctionType
ALU = mybir.AluOpType
NPBF16 = ml_dtypes.bfloat16

N_CORES = 8
B, S, C = 2, 2048, 2048
N_HEADS, D, DH = 16, 128, 64
EPS = 1e-6

# Derived tiling constants (128-partition tiles everywhere).
HL = N_HEADS // N_CORES      # local heads per core
CL = HL * D                  # local channels
KT = C // 128                # contraction tiles
ST = S // 128                # token tiles per batch
CHUNK = S // N_CORES         # a2a chunk rows per batch
TL = B * CHUNK               # local output tokens per core
SCALE = 1.0 / math.sqrt(D)

SWAP16 = [(i + 16) % 32 for i in range(32)]  # stream_shuffle half-pair swap


def _head_perm():
    """Channel permutation for q/k: within each head's 128 channels, each
    32-partition quadrant holds [16 reals | 16 imags] of 16 adjacent
    complex pairs, so the RoPE partner lives 16 partitions away."""
    perm = np.empty(128, np.int64)
    for r in range(128):
        qd, lane = divmod(r, 32)
        pair = 16 * qd + (lane % 16)
        perm[r] = 2 * pair + (1 if lane >= 16 else 0)
    return perm  # perm[r] = original within-head channel at partition r


PERM128 = _head_perm()
PAIR_OF_ROW = (PERM128 // 2)          # complex pair index per partition row
ROW_IS_IMAG = (PERM128 % 2).astype(bool)



def build_program(has_bias_qk, has_bias_v, has_g, has_mask):
    from contextlib import ExitStack

    nc = bacc.Bacc(
        "TRN2",
        target_bir_lowering=False,
        debug=False,
        enable_asserts=True,
        num_devices=N_CORES,
    )

    xT = nc.dram_tensor("xT", [C, B * S], BF16, kind="ExternalInput")
    wqT = nc.dram_tensor("wqT", [C, CL], BF16, kind="ExternalInput")
    wkT = nc.dram_tensor("wkT", [C, CL], BF16, kind="ExternalInput")
    wvT = nc.dram_tensor("wvT", [C, CL], BF16, kind="ExternalInput")
    woT = nc.dram_tensor("woT", [C, C], BF16, kind="ExternalInput")
    cosD = nc.dram_tensor("cosD", [128, S], BF16, kind="ExternalInput")
    sinD = nc.dram_tensor("sinD", [128, S], BF16, kind="ExternalInput")
    bqk = (
        nc.dram_tensor("bqk", [128, 2 * HL], F32, kind="ExternalInput")
        if has_bias_qk
        else None
    )
    bvb = (
        nc.dram_tensor("bvb", [128, CL], F32, kind="ExternalInput")
        if has_bias_v
        else None
    )
    gqk = (
        nc.dram_tensor("gqk", [128, 2 * HL], F32, kind="ExternalInput")
        if has_g
        else None
    )
    maskkT = (
        nc.dram_tensor("maskkT", [B, 128, ST], F32, kind="ExternalInput")
        if has_mask
        else None
    )
    out_loc = nc.dram_tensor("out_loc", [TL, C], F32, kind="ExternalOutput")

    groups = [list(range(N_CORES))]
    HS = S // 2  # token half per (b, th) slab

    with tile.TileContext(nc) as tc, ExitStack() as top:
        const = top.enter_context(tc.tile_pool(name="const", bufs=1))
        dram = top.enter_context(tc.tile_pool(name="dram", bufs=1, space="DRAM"))
        qkbf_p = top.enter_context(tc.tile_pool(name="qkbf", bufs=2 * 2 * HL))
        vext_p = top.enter_context(tc.tile_pool(name="vext", bufs=B * ST))

        ones_col = const.tile([128, 1], BF16)
        nc.vector.memset(ones_col[:], 1.0)
        eps_col = const.tile([2, 1], F32)
        nc.vector.memset(eps_col[:], EPS)
        if has_mask:
            maskk_sb = const.tile([128, B * ST], F32)
            nc.sync.dma_start(
                out=maskk_sb[:].rearrange("p (b t) -> p b t", b=B),
                in_=maskkT[:].rearrange("b p t -> p b t"),
            )

        # --- internal DRAM ---
        ar_in = [dram.tile([2, S], F32, name=f"ar_in{b}") for b in range(B)]
        ar_out = [dram.tile([2, S], F32, name=f"ar_out{b}") for b in range(B)]
        rs_dr = [dram.tile([2, S], F32, name=f"rs_dr{b}") for b in range(B)]
        a2a_in = [
            [
                dram.tile([N_CORES, CHUNK, D], BF16, name=f"a2a_in{b}_{hl}")
                for hl in range(HL)
            ]
            for b in range(B)
        ]
        a2a_out = [
            [
                dram.tile([N_CORES, CHUNK, D], BF16, name=f"a2a_out{b}_{hl}")
                for hl in range(HL)
            ]
            for b in range(B)
        ]

        qbf = [[None] * HL for _ in range(B)]
        kbf = [[None] * HL for _ in range(B)]
        vext = [[None] * ST for _ in range(B)]

        # ====================== QKV + norm + rope ======================
        # All pools for this phase live for the whole batch loop; slot reuse
        # (tags) creates fine-grained cross-batch deps, so batch 1's
        # projections overlap batch 0's rope/AllReduce window.
        qkvstk = ExitStack()
        xk_p = qkvstk.enter_context(tc.tile_pool(name="xk", bufs=1))
        wst_p = qkvstk.enter_context(tc.tile_pool(name="wst", bufs=1))
        raw_p = qkvstk.enter_context(tc.tile_pool(name="raw", bufs=4 * B))
        q2_p = qkvstk.enter_context(tc.tile_pool(name="q2", bufs=2))
        ss_p = qkvstk.enter_context(tc.tile_pool(name="ssb", bufs=1))
        rs_p = qkvstk.enter_context(tc.tile_pool(name="rs", bufs=2))
        cs_p = qkvstk.enter_context(tc.tile_pool(name="cs", bufs=1))
        rope_p = qkvstk.enter_context(tc.tile_pool(name="rope", bufs=1))
        # NOTE: rs/cs/rope intentionally opened last: they release latest
        # (rope of batch 1), and the attention pt pool below must land on the
        # early-released xk/raw zones instead.
        qkv_psum = ExitStack()
        qkps = qkv_psum.enter_context(tc.tile_pool(name="qkps", bufs=2, space="PSUM"))
        vps = qkv_psum.enter_context(tc.tile_pool(name="vps", bufs=2, space="PSUM"))
        ssps = qkv_psum.enter_context(tc.tile_pool(name="ssps", bufs=1, space="PSUM"))

        wvr = cs_p.tile([128, KT * CL], BF16)
        nc.sync.dma_start(
            out=wvr[:].rearrange("p (kt c) -> p kt c", kt=KT),
            in_=wvT[:].rearrange("(kt p) c -> p kt c", p=128),
        )
        cos_sb = cs_p.tile([128, S], BF16)
        sin_sb = cs_p.tile([128, S], BF16)
        nc.sync.dma_start(out=cos_sb[:], in_=cosD[:])
        nc.sync.dma_start(out=sin_sb[:], in_=sinD[:])
        if has_bias_qk:
            bqk_sb = cs_p.tile([128, 2 * HL], F32)
            nc.sync.dma_start(out=bqk_sb[:], in_=bqk[:])
        if has_bias_v:
            bvb_sb = cs_p.tile([128, CL], F32)
            nc.sync.dma_start(out=bvb_sb[:], in_=bvb[:])
        if has_g:
            gqk_sb = cs_p.tile([128, 2 * HL], F32)
            nc.sync.dma_start(out=gqk_sb[:], in_=gqk[:])

        raws = []
        for b in range(B):
            raw = {}
            for tname in ("q", "k"):
                for ct in range(HL):
                    raw[(tname, ct)] = raw_p.tile(
                        [128, S], BF16, name=f"raw{tname}{b}_{ct}", tag="raw"
                    )
            raws.append(raw)

        def emit_xk(b, th):
            xk = xk_p.tile(
                [128, KT * HS], BF16, name=f"xk{b}{th}", tag="xk", bufs=2
            )
            for kt in range(KT):
                nc.sync.dma_start(
                    out=xk[:, kt * HS : (kt + 1) * HS],
                    in_=xT[
                        kt * 128 : (kt + 1) * 128,
                        b * S + th * HS : b * S + (th + 1) * HS,
                    ],
                )
            return xk

        def emit_qk(b, th, xk):
            raw = raws[b]
            # ---- Q then K projections (channel-major) ----
            for tname, w_dr in (("q", wqT), ("k", wkT)):
                ps = {
                    (ct, sl): qkps.tile(
                        [128, 512], F32,
                        name=f"ps{tname}{b}{th}{ct}{sl}", tag="qkps", bufs=5,
                    )
                    for ct in range(HL)
                    for sl in range(HS // 512)
                }
                for kt in range(KT):
                    wt = wst_p.tile(
                        [128, CL], BF16, name=f"w{tname}{b}{th}{kt}", tag="wst",
                        bufs=4,
                    )
                    nc.sync.dma_start(
                        out=wt[:], in_=w_dr[kt * 128 : (kt + 1) * 128, :]
                    )
                    for ct in range(HL):
                        for sl in range(HS // 512):
                            nc.tensor.matmul(
                                ps[(ct, sl)][:],
                                wt[:, ct * 128 : (ct + 1) * 128],
                                xk[:, kt * HS + sl * 512 : kt * HS + (sl + 1) * 512],
                                start=(kt == 0),
                                stop=(kt == KT - 1),
                            )
                for ct in range(HL):
                    col = ct + (0 if tname == "q" else HL)
                    for sl in range(HS // 512):
                        dst = raw[(tname, ct)][
                            :, th * HS + sl * 512 : th * HS + (sl + 1) * 512
                        ]
                        if has_bias_qk:
                            nc.scalar.activation(
                                dst, ps[(ct, sl)][:], AF.Copy,
                                bias=bqk_sb[:, col : col + 1],
                            )
                        else:
                            nc.vector.tensor_copy(dst, ps[(ct, sl)][:])
            # ---- per-token sum-of-squares partials ----
            for tname in ("q", "k"):
                q2s = []
                for ct in range(HL):
                    q2 = q2_p.tile(
                        [128, HS], BF16, name=f"q2{tname}{b}{th}{ct}", tag="q2"
                    )
                    src2 = raw[(tname, ct)][:, th * HS : (th + 1) * HS]
                    nc.vector.tensor_tensor(q2[:], src2, src2, ALU.mult)
                    q2s.append(q2)
                row = 0 if tname == "q" else 1
                for sl in range(HS // 512):
                    pss = ssps.tile(
                        [1, 512], F32, name=f"pss{tname}{b}{th}{sl}", tag="ssps",
                        bufs=1,
                    )
                    for ct in range(HL):
                        nc.tensor.matmul(
                            pss[:],
                            ones_col[:],
                            q2s[ct][:, sl * 512 : (sl + 1) * 512],
                            start=(ct == 0),
                            stop=(ct == HL - 1),
                        )
                    sss = ss_p.tile(
                        [1, 512], F32, name=f"sss{tname}{b}{th}{sl}", tag="sss",
                        bufs=3,
                    )
                    nc.scalar.activation(sss[:], pss[:], AF.Copy)
                    nc.sync.dma_start(
                        out=ar_in[b][
                            row, th * HS + sl * 512 : th * HS + (sl + 1) * 512
                        ],
                        in_=sss[:],
                    )

        def emit_v(b, th, xk):
            # ---- V projection (token-major) ----
            for tt8 in range(ST // 2):
                tt = th * (ST // 2) + tt8
                psv = vps.tile(
                    [128, CL], F32, name=f"psv{b}{th}{tt8}", tag="vps"
                )
                vx = vext_p.tile(
                    [128, HL * 129], BF16, name=f"vx{b}_{tt}", tag="vx"
                )
                vext[b][tt] = vx
                nc.vector.memset(vx[:], 1.0)
                for kt in range(KT):
                    nc.tensor.matmul(
                        psv[:],
                        xk[:, kt * HS + tt8 * 128 : kt * HS + tt8 * 128 + 128],
                        wvr[:, kt * CL : (kt + 1) * CL],
                        start=(kt == 0),
                        stop=(kt == KT - 1),
                    )
                for hl in range(HL):
                    dstv = vx[:, hl * 129 : hl * 129 + 128]
                    srcv = psv[:, hl * 128 : (hl + 1) * 128]
                    if has_bias_v:
                        nc.vector.scalar_tensor_tensor(
                            dstv,
                            srcv,
                            1.0,
                            bvb_sb[:, hl * 128 : (hl + 1) * 128],
                            ALU.mult,
                            ALU.add,
                        )
                    else:
                        nc.vector.tensor_copy(dstv, srcv)

        def emit_rs_rope(b):
            raw = raws[b]
            # ---- rsqrt chain + broadcast ----
            # rsqrt(mean + eps) = exp(-0.5 * ln(sumsq/C + eps)), in place.
            ss2 = ss_p.tile([2, S], F32, name=f"ss2_{b}", tag="ssw", bufs=1)
            nc.sync.dma_start(out=ss2[:], in_=ar_out[b][:])
            nc.scalar.activation(
                ss2[:], ss2[:], AF.Ln, scale=1.0 / C, bias=eps_col[:]
            )
            nc.scalar.activation(ss2[:], ss2[:], AF.Exp, scale=-0.5)
            nc.sync.dma_start(out=rs_dr[b][:], in_=ss2[:])

            rs_b = {}
            for row, tname in ((0, "q"), (1, "k")):
                rt = rs_p.tile([128, S], F32, name=f"rs{tname}{b}", tag="rs")
                nc.sync.dma_start(
                    out=rt[:],
                    in_=rs_dr[b][row : row + 1, :].to_broadcast([128, S]),
                )
                rs_b[tname] = rt

            # ---- rope (on raw, AR-independent) then rmsnorm scale last ----
            # Half-width working tiles with 2 slots per tag: two tiles
            # pipeline on DVE and the rs-dependent final multiply starts as
            # soon as the first half is rotated.
            dsts = {}
            for tname, dstarr in (("q", qbf), ("k", kbf)):
                for ct in range(HL):
                    dst = qkbf_p.tile([128, S], BF16, name=f"bf{b}{tname}{ct}",
                                      tag="qkbf")
                    dsts[(tname, ct)] = dst
                    dstarr[b][ct] = dst
            for th in range(2):
                sl_ = slice(th * HS, (th + 1) * HS)
                for tname in ("q", "k"):
                    for ct in range(HL):
                        src = raw[(tname, ct)]
                        if has_g:
                            col = ct + (0 if tname == "q" else HL)
                            gsrc = rope_p.tile(
                                [128, HS], BF16, name=f"g{b}{tname}{ct}{th}",
                                tag="gsrc", bufs=2,
                            )
                            nc.vector.tensor_scalar_mul(
                                gsrc[:], src[:, sl_], gqk_sb[:, col : col + 1]
                            )
                            srcv = gsrc[:]
                        else:
                            srcv = src[:, sl_]
                        ysw = rope_p.tile(
                            [128, HS], BF16, name=f"ysw{b}{tname}{ct}{th}",
                            tag="ysw", bufs=2,
                        )
                        nc.vector.stream_shuffle(ysw[:], srcv, SWAP16)
                        # In-place: raw *= cos, ysw *= sin, dst = raw + ysw,
                        # dst *= rs.  raw is dead after this.
                        nc.vector.tensor_tensor(
                            srcv, srcv, cos_sb[:, sl_], ALU.mult
                        )
                        nc.vector.tensor_tensor(
                            ysw[:], ysw[:], sin_sb[:, sl_], ALU.mult
                        )
                        dslc = dsts[(tname, ct)][:, sl_]
                        nc.vector.tensor_tensor(dslc, srcv, ysw[:], ALU.add)
                        nc.vector.tensor_tensor(
                            dslc, dslc, rs_b[tname][:, sl_], ALU.mult
                        )

        # Emission order drives engine priority: per batch, q/k projections
        # and their sum-of-squares go first so the rmsnorm AllReduce fires as
        # early as possible; V then runs on the still-resident x slabs while
        # the collective is in flight, and rope follows.  Batch 1's chain
        # (AR -> rope -> attention -> a2a -> wo) is the critical path.
        for b in range(B):
            for th in range(2):
                xk = emit_xk(b, th)
                emit_qk(b, th, xk)
                emit_v(b, th, xk)
            nc.gpsimd.collective_compute(
                "AllReduce",
                ALU.add,
                replica_groups=groups,
                ins=[ar_in[b][:].opt()],
                outs=[ar_out[b][:].opt()],
            )
            emit_rs_rope(b)
        qkvstk.close()

        qkv_psum.close()


        # ====================== attention + output projection ======================
        with ExitStack() as astk:
            attn_psum = ExitStack()
            stps = attn_psum.enter_context(
                tc.tile_pool(name="stps", bufs=3, space="PSUM")
            )
            pvps = attn_psum.enter_context(
                tc.tile_pool(name="pvps", bufs=2, space="PSUM")
            )
            pt_pa = astk.enter_context(tc.tile_pool(name="pt_a", bufs=17))
            pt_pb = astk.enter_context(tc.tile_pool(name="pt_b", bufs=18))
            pt_half_n = [0]
            rec_p = astk.enter_context(tc.tile_pool(name="rec", bufs=4))
            abf_p = astk.enter_context(tc.tile_pool(name="abf", bufs=4))
            at_p = astk.enter_context(tc.tile_pool(name="at", bufs=B * KT))
            wo_p = astk.enter_context(tc.tile_pool(name="wo", bufs=KT))
            osb_p = astk.enter_context(tc.tile_pool(name="osb", bufs=3))

            at_tiles = [[None] * KT for _ in range(B)]

            for b in range(B):
                for hl in range(HL):
                    qh = qbf[b][hl]
                    kh = kbf[b][hl]
                    for H in range(2):
                        pool = pt_pa if pt_half_n[0] % 2 == 0 else pt_pb
                        pt_half_n[0] += 1
                        pts = []
                        for tk in range(ST):
                            pt = pool.tile([128, S // 2], BF16,
                                           name=f"pt{b}{hl}{H}_{tk}", tag="pt")
                            pts.append(pt)
                            pss = stps.tile(
                                [128, 1024], F32, name=f"st{b}{hl}{H}{tk}", tag="st"
                            )
                            for sl in range(2):
                                nc.tensor.matmul(
                                    pss[:, sl * 512 : (sl + 1) * 512],
                                    kh[:, tk * 128 : (tk + 1) * 128],
                                    qh[
                                        :,
                                        (H * 2 + sl) * 512 : (H * 2 + sl + 1) * 512,
                                    ],
                                    start=True,
                                    stop=True,
                                )
                            nc.scalar.activation(
                                pt[:], pss[:], AF.Exp, scale=SCALE
                            )
                            if has_mask:
                                nc.vector.tensor_scalar_mul(
                                    pt[:],
                                    pt[:],
                                    maskk_sb[:, b * ST + tk : b * ST + tk + 1],
                                )
                        for tq8 in range(ST // 2):
                            tq = H * (ST // 2) + tq8
                            po = pvps.tile([128, 129], F32, name=f"po{b}{hl}{tq}",
                                           tag="po")
                            for tk in range(ST):
                                nc.tensor.matmul(
                                    po[:],
                                    pts[tk][:, tq8 * 128 : (tq8 + 1) * 128],
                                    vext[b][tk][:, hl * 129 : (hl + 1) * 129],
                                    start=(tk == 0),
                                    stop=(tk == ST - 1),
                                )
                            rec = rec_p.tile([128, 1], F32, name=f"rec{b}{hl}{tq}",
                                             tag="rec")
                            nc.vector.reciprocal(rec[:], po[:, 128:129])
                            abf = abf_p.tile([128, D], BF16, name=f"abf{b}{hl}{tq}",
                                             tag="abf")
                            nc.vector.tensor_scalar_mul(abf[:], po[:, 0:D], rec[:])
                            j, r0 = divmod(tq * 128, CHUNK)
                            nc.sync.dma_start(
                                out=a2a_in[b][hl][j, r0 : r0 + 128, :],
                                in_=abf[:],
                            )

                    # Per-head AllToAll: head 0's resharding is in flight
                    # while head 1's attention still computes, and the wo
                    # contraction starts on head-0 columns before head 1
                    # arrives.
                    nc.gpsimd.collective_compute(
                        "AllToAll",
                        ALU.bypass,
                        replica_groups=groups,
                        ins=[a2a_in[b][hl][:].opt()],
                        outs=[a2a_out[b][hl][:].opt()],
                    )
                    for i in range(N_CORES):
                        ci = i * HL + hl
                        at = at_p.tile([128, CHUNK], BF16, name=f"at{b}_{ci}",
                                       tag="at")
                        at_tiles[b][ci] = at
                        nc.sync.dma_start_transpose(
                            at[:], a2a_out[b][hl][i, :, :]
                        )

            # Score/PV PSUM banks are dead once attention finishes; reuse
            # them for a wider wo accumulation pool for batch 1 (whose wo
            # runs after attention anyway), so its chains run in parallel.
            attn_psum.close()
            wops2 = astk.enter_context(
                tc.tile_pool(name="wops2", bufs=3, space="PSUM")
            )
            ci_order = [i * HL + 0 for i in range(N_CORES)] + [
                i * HL + 1 for i in range(N_CORES)
            ]
            # wo weights are batch-independent: load both column halves once
            # up front (prefetched during attention) and reuse for b0 and b1.
            wo_sb_all = []
            for half in range(2):
                wo_sb = []
                for ci in range(KT):
                    wt = wo_p.tile(
                        [128, C // 2], BF16, name=f"wo{half}{ci}", tag="wo",
                        bufs=2 * KT,
                    )
                    nc.sync.dma_start(
                        out=wt[:],
                        in_=woT[
                            ci * 128 : (ci + 1) * 128,
                            half * (C // 2) : (half + 1) * (C // 2),
                        ],
                    )
                    wo_sb.append(wt)
                wo_sb_all.append(wo_sb)
            for b in range(B):
                for half in range(2):
                    wo_sb = wo_sb_all[half]
                    for tt in range(CHUNK // 128):
                        pso = [
                            wops2.tile(
                                [128, 512], F32, name=f"pso{b}{half}{tt}{q}",
                                tag="pso",
                            )
                            for q in range(C // 1024)
                        ]
                        for cn, ci in enumerate(ci_order):
                            lhsT = at_tiles[b][ci][:, tt * 128 : (tt + 1) * 128]
                            for q in range(C // 1024):
                                nc.tensor.matmul(
                                    pso[q][:],
                                    lhsT,
                                    wo_sb[ci][:, q * 512 : (q + 1) * 512],
                                    start=(cn == 0),
                                    stop=(cn == KT - 1),
                                )
                        for q in range(C // 1024):
                            osb = osb_p.tile(
                                [128, 512], F32, name=f"osb{b}{half}{tt}{q}",
                                tag="osb",
                            )
                            nc.scalar.activation(osb[:], pso[q][:], AF.Copy)
                            nc.sync.dma_start(
                                out=out_loc[
                                    b * CHUNK + tt * 128 : b * CHUNK + (tt + 1) * 128,
                                    half * (C // 2)
                                    + q * 512 : half * (C // 2)
                                    + (q + 1) * 512,
                                ],
                                in_=osb[:],
                            )

    nc.compile()
    return nc



def _rope_volume_np(freqs_cs, f_p, h_p, w_p):
    t_dim = DH - 2 * (DH // 3)
    s_dim = DH // 3
    a_cos = np.asarray(freqs_cs[..., 0], np.float32)
    a_sin = np.asarray(freqs_cs[..., 1], np.float32)

    def vol(a):
        at = np.broadcast_to(a[:f_p, None, None, :t_dim], (f_p, h_p, w_p, t_dim))
        ah = np.broadcast_to(
            a[None, :h_p, None, t_dim : t_dim + s_dim], (f_p, h_p, w_p, s_dim)
        )
        aw = np.broadcast_to(
            a[None, None, :w_p, t_dim + s_dim :], (f_p, h_p, w_p, s_dim)
        )
        return np.concatenate([at, ah, aw], axis=-1).reshape(f_p * h_p * w_p, DH)

    return vol(a_cos), vol(a_sin)


_PROGRAM_CACHE = {}
_RUNNER_CACHE = {}


def _make_runner(nc):
    """Build a cached jitted shard_map runner for the compiled Bass program.

    Mirrors bass2jax.run_bass_via_pjrt but keeps the jitted function and lets
    the caller reuse device-resident input buffers for steady-state timing.
    """
    import jax
    from jax.sharding import Mesh, NamedSharding, PartitionSpec
    from jax.experimental.shard_map import shard_map
    import concourse.mybir as _mybir
    from concourse.bass2jax import (
        _bass_exec_p,
        install_neuronx_cc_hook,
        partition_id_tensor,
    )

    install_neuronx_cc_hook()
    partition_name = nc.partition_id_tensor.name if nc.partition_id_tensor else None

    in_names, out_names, out_avals = [], [], []
    zero_outs = []
    for alloc in nc.m.functions[0].allocations:
        if not isinstance(alloc, _mybir.MemoryLocationSet):
            continue
        name = alloc.memorylocations[0].name
        if alloc.kind == "ExternalInput":
            if name != partition_name:
                in_names.append(name)
        elif alloc.kind == "ExternalOutput":
            shape = tuple(alloc.tensor_shape)
            dtype = _mybir.dt.np(alloc.dtype)
            out_names.append(name)
            out_avals.append(jax.core.ShapedArray(shape, dtype))
            zero_outs.append(np.zeros(shape, dtype))
    n_params = len(in_names)
    all_in_names = list(in_names) + list(out_names)
    if partition_name is not None:
        all_in_names.append(partition_name)

    def _body(*args):
        operands = list(args)
        if partition_name is not None:
            operands.append(partition_id_tensor())
        outs = _bass_exec_p.bind(
            *operands,
            out_avals=tuple(out_avals),
            in_names=tuple(all_in_names),
            out_names=tuple(out_names),
            lowering_input_output_aliases=(),
            sim_require_finite=True,
            sim_require_nnan=True,
            nc=nc,
        )
        return tuple(outs)

    devices = jax.devices()[:N_CORES]
    mesh = Mesh(np.asarray(devices), ("core",))
    psharding = NamedSharding(mesh, PartitionSpec("core"))
    nin = n_params + len(out_names)
    sharded = jax.jit(
        shard_map(
            _body,
            mesh=mesh,
            in_specs=(PartitionSpec("core"),) * nin,
            out_specs=(PartitionSpec("core"),) * len(out_names),
            check_rep=False,
        ),
        keep_unused=True,
    )

    def run(in_maps, timing_iters=0):
        per_core = [[np.asarray(m[nm]) for nm in in_names] for m in in_maps]
        concat_in = [
            np.concatenate([per_core[c][i] for c in range(N_CORES)], axis=0)
            for i in range(n_params)
        ]
        concat_zeros = [
            np.zeros((N_CORES * z.shape[0], *z.shape[1:]), z.dtype)
            for z in zero_outs
        ]
        args = [
            jax.device_put(a, psharding) for a in (*concat_in, *concat_zeros)
        ]
        out_arrs = sharded(*args)
        jax.block_until_ready(out_arrs)
        best_ns = None
        if timing_iters:
            import time as _time

            # Burst timing: issue `timing_iters` executions back-to-back and
            # block once.  The device runs them serially, so total/amortized
            # per-call time measures actual steady-state execution without
            # counting the client<->terminal network round-trip once per call.
            for _ in range(5):
                t0 = _time.perf_counter()
                outs = [sharded(*args) for _ in range(timing_iters)]
                jax.block_until_ready(outs)
                dt = (_time.perf_counter() - t0) * 1e9 / timing_iters
                best_ns = dt if best_ns is None else min(best_ns, dt)
                del outs
        results = [
            {
                name: np.asarray(out_arrs[i]).reshape(N_CORES, *out_avals[i].shape)[c]
                for i, name in enumerate(out_names)
            }
            for c in range(N_CORES)
        ]
        return results, best_ns

    return run


def kernel(
    x,
    freqs_cs,
    wq,
    bq,
    wk,
    bk,
    wv,
    bv,
    wo,
    bo,
    gq,
    gk,
    frame_mask,
    f_p,
    h_p,
    w_p,
):
    x = np.asarray(x, np.float32)
    freqs_cs = np.asarray(freqs_cs, np.float32)
    wq, wk, wv, wo = (np.asarray(w, np.float32) for w in (wq, wk, wv, wo))
    bq, bk, bv, bo = (np.asarray(v, np.float32) for v in (bq, bk, bv, bo))
    gq, gk = np.asarray(gq, np.float32), np.asarray(gk, np.float32)
    mask = np.asarray(frame_mask, bool)
    f_p, h_p, w_p = int(f_p), int(h_p), int(w_p)

    has_bias_qk = bool(np.any(bq) or np.any(bk))
    has_bias_v = bool(np.any(bv))
    has_g = not (np.all(gq == 1.0) and np.all(gk == 1.0))
    has_mask = not bool(mask.all())

    key = (has_bias_qk, has_bias_v, has_g, has_mask)
    if key not in _PROGRAM_CACHE:
        _PROGRAM_CACHE[key] = build_program(*key)
    nc = _PROGRAM_CACHE[key]

    # ---------------- host-side prep ----------------
    cos_vol, sin_vol = _rope_volume_np(freqs_cs, f_p, h_p, w_p)  # [S, DH]
    cosD = cos_vol[:, PAIR_OF_ROW].T.astype(np.float32).copy()  # [128, S]
    sinD = sin_vol[:, PAIR_OF_ROW].T.astype(np.float32).copy()
    sinD[~ROW_IS_IMAG, :] *= -1.0
    cosD = cosD.astype(NPBF16)
    sinD = sinD.astype(NPBF16)

    xT = np.ascontiguousarray(x.reshape(B * S, C).T).astype(NPBF16)
    woT = np.ascontiguousarray(wo.T).astype(NPBF16)

    in_maps = []
    for core in range(N_CORES):
        ch0 = core * CL
        qk_rows = np.concatenate(
            [ch0 + hl * D + PERM128 for hl in range(HL)]
        )  # permuted global channels for q/k
        v_rows = np.arange(ch0, ch0 + CL)
        m = {
            "xT": xT,
            "wqT": np.ascontiguousarray(wq[qk_rows, :].T).astype(NPBF16),
            "wkT": np.ascontiguousarray(wk[qk_rows, :].T).astype(NPBF16),
            "wvT": np.ascontiguousarray(wv[v_rows, :].T).astype(NPBF16),
            "woT": woT,
            "cosD": cosD,
            "sinD": sinD,
        }
        if has_bias_qk:
            bq_l = bq[qk_rows].reshape(HL, 128).T
            bk_l = bk[qk_rows].reshape(HL, 128).T
            m["bqk"] = np.ascontiguousarray(
                np.concatenate([bq_l, bk_l], axis=1)
            ).astype(np.float32)
        if has_bias_v:
            m["bvb"] = np.ascontiguousarray(
                np.broadcast_to(bv[v_rows][None, :], (128, CL))
            ).astype(np.float32)
        if has_g:
            gq_l = gq[qk_rows].reshape(HL, 128).T
            gk_l = gk[qk_rows].reshape(HL, 128).T
            m["gqk"] = np.ascontiguousarray(
                np.concatenate([gq_l, gk_l], axis=1)
            ).astype(np.float32)
        if has_mask:
            mk = mask.astype(np.float32).reshape(B, ST, 128).transpose(0, 2, 1)
            m["maskkT"] = np.ascontiguousarray(mk)
        in_maps.append(m)

    if key not in _RUNNER_CACHE:
        _RUNNER_CACHE[key] = _make_runner(nc)
    timing_iters = int(os.environ.get("ATTN_TIME_ITERS", "0"))
    results, best_ns = _RUNNER_CACHE[key](in_maps, timing_iters=timing_iters)
    kernel._last_time_ns = best_ns

    out = np.empty((B * S, C), np.float32)
    for core in range(N_CORES):
        o = results[core]["out_loc"]
        for b in range(B):
            out[b * S + core * CHUNK : b * S + (core + 1) * CHUNK, :] = o[
                b * CHUNK : (b + 1) * CHUNK, :
            ]
    if np.any(bo):
        out += bo[None, :]
    out = out.reshape(B, S, C)
    if has_mask:
        out = np.where(mask[:, :, None], out, 0.0)
    return out
